# revision 14
# baseline (speedup 1.0000x reference)
"""Trainium2 Bass kernel for nn_ContrastivePhaseObjective.

Strategy: token-locality sharding over LIVE columns + host pre-norm.

The reference loss depends only on sims[k, n] where tok[n] == ta[k] and
n != anchor_k.  Sorting positions by token id makes each anchor's
candidate set a contiguous run; the union of runs over the <=256 anchor
tokens covers ~1.3k of the 65k positions ("live" columns).  Each core
gets a contiguous slice of the live-column list (padded to LIVE) plus
the <=128 anchors whose runs intersect its slice, and computes the
dense masked cosine-sim reduction over its slice on device:

  - x   [128, 4, LIVE] f16: normalized live columns (d-block x {r,i})
  - a   [128, 4, 128]  f16: normalized anchors (scaled by 1/|a|)
  - eqm [128, LIVE]    f16 in {0,-3}: 0 = valid other, -3 = invalid /
                        self / dead slot or pad column
  - PE: psum = 4 accumulating f16 matmuls (contraction 256 x {r,i})
  - ACT: c16 = f16 copy of psum (sims)
  - DVE: pacc = c16 + eqm (column-max partial), nacc = c16 - eqm
         (column-min partial); running max/min across chunks
  - out [128, 2, 512] f16 partials -> host

Host combines per-anchor max/min across cores/columns (runs split
across cores are handled by the max/min union) and applies the loss
formula with exact integer index math.  If an input ever needs more
than LIVE live columns per core, a wider program from the size ladder
is compiled (up to the fully dense 8192).
"""

from contextlib import ExitStack

import numpy as np

import concourse.bacc as bacc
import concourse.tile as tile
from concourse import mybir
from concourse.bass_utils import run_bass_kernel_spmd

# ---- problem constants (hardcoded per harness contract) ----
B, S, D = 16, 4096, 256
N = B * S
VOCAB = 16000
KMAX = 256  # MAX_ANCHORS
EPS = 1e-8
TEMPERATURE = 0.1
MARGIN = 1.0
SEPARATION_WEIGHT = 1.0
NCORES = 8

CHUNK = 192
LIVE = 192  # padded live columns per core (harness input needs ~170)
KC = 64  # anchor slots per core (harness input needs <=35)
MASK = 3.0  # additive mask magnitude

F32 = mybir.dt.float32
F16 = mybir.dt.float16

_PROGRAM_CACHE = {}


def build_program(w):
    """Masked-sim partial-reduction program over a width-w column slice."""
    nch = (w + CHUNK - 1) // CHUNK
    assert w % CHUNK == 0
    nc = bacc.Bacc("TRN2", target_bir_lowering=False, debug=False, num_devices=NCORES)
    x_d = nc.dram_tensor("x", [nch, 128, 4, CHUNK], F16, kind="ExternalInput")
    eqm_d = nc.dram_tensor("eqm", [nch, KC, CHUNK], F16, kind="ExternalInput")
    a_d = nc.dram_tensor("a", [128, 4, KC], F16, kind="ExternalInput")
    out_d = nc.dram_tensor("out", [KC, 2, CHUNK], F16, kind="ExternalOutput")

    with tile.TileContext(nc) as tc, ExitStack() as ctx:
        singles = ctx.enter_context(tc.tile_pool(name="singles", bufs=1))
        xpool = ctx.enter_context(tc.tile_pool(name="xpool", bufs=min(nch, 6)))
        eqpool = ctx.enter_context(tc.tile_pool(name="eqpool", bufs=min(nch, 6)))
        scrpool = ctx.enter_context(tc.tile_pool(name="scrpool", bufs=4))
        pspool = ctx.enter_context(tc.tile_pool(name="pspool", bufs=4, space="PSUM"))

        # input DMAs on separate trigger engines so transfers overlap;
        # x (largest, first consumer) goes on sync, which triggers earliest
        acc = singles.tile([KC, 2, CHUNK], F16)
        xts, eqts = [], []
        for ch in range(nch):
            xt = xpool.tile([128, 4, CHUNK], F16, tag="x", name=f"x{ch}")
            nc.sync.dma_start(out=xt, in_=x_d[ch])
            eqt = eqpool.tile([KC, CHUNK], F16, tag="eq", name=f"eq{ch}")
            nc.gpsimd.dma_start(out=eqt, in_=eqm_d[ch])
            xts.append(xt)
            eqts.append(eqt)
        a_sb = singles.tile([128, 4, KC], F16)
        nc.scalar.dma_start(out=a_sb, in_=a_d[:, :, :])

        for ch in range(nch):
            xt, eqt = xts[ch], eqts[ch]
            pst = pspool.tile([KC, CHUNK], F32, tag="ps")
            for b in range(4):
                nc.tensor.matmul(
                    pst,
                    a_sb[:, b, :],
                    xt[:, b, :],
                    start=(b == 0),
                    stop=(b == 3),
                )
            if ch == 0:
                nc.vector.tensor_tensor(
                    out=acc[:, 0, :], in0=pst, in1=eqt, op=mybir.AluOpType.add
                )
                if nch == 1:
                    # overlap the pos-partial writeback with the neg chain
                    nc.sync.dma_start(out=out_d[:, 0, :], in_=acc[:, 0, :])
                nc.vector.tensor_tensor(
                    out=acc[:, 1, :], in0=pst, in1=eqt, op=mybir.AluOpType.subtract
                )
                if nch == 1:
                    nc.scalar.dma_start(out=out_d[:, 1, :], in_=acc[:, 1, :])
            else:
                scr = scrpool.tile([KC, CHUNK], F16, tag="scr")
                nc.vector.tensor_tensor(
                    out=scr, in0=pst, in1=eqt, op=mybir.AluOpType.add
                )
                nc.vector.tensor_tensor(
                    out=acc[:, 0, :], in0=acc[:, 0, :], in1=scr,
                    op=mybir.AluOpType.max,
                )
                scr2 = scrpool.tile([KC, CHUNK], F16, tag="scr2")
                nc.vector.tensor_tensor(
                    out=scr2, in0=pst, in1=eqt, op=mybir.AluOpType.subtract
                )
                nc.vector.tensor_tensor(
                    out=acc[:, 1, :], in0=acc[:, 1, :], in1=scr2,
                    op=mybir.AluOpType.min,
                )
        if nch > 1:
            nc.sync.dma_start(out=out_d[:, :, :], in_=acc)

    nc.compile()
    return nc


def host_prep(real_embeds, imag_embeds, token_ids):
    """Live-column selection, normalization, per-core input build."""
    Rf = np.asarray(real_embeds, dtype=np.float32).reshape(N, D)
    If = np.asarray(imag_embeds, dtype=np.float32).reshape(N, D)
    tok = np.asarray(token_ids).reshape(N).astype(np.int64, copy=False)

    counts = np.bincount(tok, minlength=VOCAB)
    repeated = counts[tok] >= 2
    order = np.argsort(~repeated, kind="stable")
    anchors = order[:KMAX]
    anchor_ok = repeated[anchors]
    ta = tok[anchors]
    num_others = counts[ta] - 1
    pair_ok = anchor_ok & (num_others >= 2)

    perm = np.argsort(tok, kind="stable")
    tok_s = tok[perm]

    # live columns: sorted positions whose token is an anchor token
    live_mask = np.zeros(VOCAB, dtype=bool)
    live_mask[ta] = True
    live_idx = np.nonzero(live_mask[tok_s])[0]  # indices into sorted order
    n_live = len(live_idx)
    per_core = -(-n_live // NCORES)  # ceil split
    w = LIVE
    while per_core > w:
        w *= 2
    assert w <= N // NCORES

    live_cols = perm[live_idx]  # original position of each live column
    live_tok = tok_s[live_idx]

    # normalize only what the device needs: live columns + anchors
    need = np.concatenate([live_cols, anchors])
    inv_need = 1.0 / np.sqrt(
        np.einsum("nd,nd->n", Rf[need], Rf[need])
        + np.einsum("nd,nd->n", If[need], If[need])
        + EPS
    )
    inv_live = inv_need[:n_live].astype(np.float32)
    inv_anc = inv_need[n_live:].astype(np.float32)

    nch = w // CHUNK
    in_maps = []
    slot_maps = []
    for c in range(NCORES):
        lo = min(c * per_core, n_live)
        hi = min(lo + per_core, n_live)
        cols = live_cols[lo:hi]
        ctok = live_tok[lo:hi]
        m = len(cols)

        xw = np.zeros((128, 4, w), dtype=np.float16)
        if m:
            Rn = (Rf[cols] * inv_live[lo:hi][:, None]).astype(np.float16)
            In = (If[cols] * inv_live[lo:hi][:, None]).astype(np.float16)
            xw[:, 0, :m] = Rn.T[:128]
            xw[:, 1, :m] = Rn.T[128:]
            xw[:, 2, :m] = In.T[:128]
            xw[:, 3, :m] = In.T[128:]
        x = np.ascontiguousarray(
            xw.reshape(128, 4, nch, CHUNK).transpose(2, 0, 1, 3)
        )

        # anchors owned: any live column here carries their token
        own = np.nonzero(np.isin(ta, ctok))[0] if m else np.array([], dtype=int)
        assert len(own) <= KC, f"core {c}: {len(own)} anchors > {KC} slots"
        slot_maps.append(own)

        a = np.zeros((128, 4, KC), dtype=np.float16)
        if len(own):
            Ra = Rf[anchors[own]] * inv_anc[own][:, None]
            Ia = If[anchors[own]] * inv_anc[own][:, None]
            a[:, 0, : len(own)] = Ra.T[:128].astype(np.float16)
            a[:, 1, : len(own)] = Ra.T[128:].astype(np.float16)
            a[:, 2, : len(own)] = Ia.T[:128].astype(np.float16)
            a[:, 3, : len(own)] = Ia.T[128:].astype(np.float16)

        eqm = np.full((KC, w), -MASK, dtype=np.float16)
        if len(own):
            valid = ta[own][:, None] == ctok[None, :]
            # self-exclusion: an anchor's own position is not an "other"
            selfcol = anchors[own][:, None] == cols[None, :]
            eqm[: len(own), :m] = np.where(valid & ~selfcol, 0.0, -MASK)
        eqm = np.ascontiguousarray(eqm.reshape(KC, nch, CHUNK).transpose(1, 0, 2))

        in_maps.append({"x": x, "eqm": eqm, "a": a})

    meta = {"pair_ok": pair_ok, "slot_maps": slot_maps, "w": w}
    return in_maps, meta


def combine(results, meta):
    pos = np.full(KMAX, -np.inf, dtype=np.float64)
    neg = np.full(KMAX, np.inf, dtype=np.float64)
    for c, res in enumerate(results):
        o = np.asarray(res["out"], dtype=np.float64)  # [128, 2, CHUNK]
        own = meta["slot_maps"][c]
        if len(own) == 0:
            continue
        p = o[: len(own), 0, :].max(axis=1)
        q = o[: len(own), 1, :].min(axis=1)
        np.maximum.at(pos, own, p)
        np.minimum.at(neg, own, q)

    pair_ok = meta["pair_ok"]
    num_pairs = int(pair_ok.sum())
    if num_pairs == 0:
        return np.float32(0.0)
    lp = pos / TEMPERATURE
    ln = neg / TEMPERATURE
    m = np.maximum(lp, ln)
    lse = m + np.log(np.exp(lp - m) + np.exp(ln - m))
    ce = lse - lp
    sep = np.maximum(neg + MARGIN, 0.0)
    per_anchor = ce + SEPARATION_WEIGHT * sep
    total = float(np.sum(per_anchor[pair_ok]))
    return np.float32(total / num_pairs)


def kernel_with_results(real_embeds, imag_embeds, token_ids, trace=False):
    in_maps, meta = host_prep(real_embeds, imag_embeds, token_ids)
    w = meta["w"]
    if w not in _PROGRAM_CACHE:
        _PROGRAM_CACHE[w] = build_program(w)
    nc = _PROGRAM_CACHE[w]
    br = run_bass_kernel_spmd(nc, in_maps, core_ids=list(range(NCORES)), trace=trace)
    loss = combine(br.results, meta)
    return loss, br


def kernel(real_embeds, imag_embeds, token_ids):
    loss, _ = kernel_with_results(real_embeds, imag_embeds, token_ids)
    return loss


# revision 15
# speedup vs baseline: 1.0008x; 1.0008x over previous
"""Trainium2 Bass kernel for nn_ContrastivePhaseObjective.

Strategy: token-locality sharding over LIVE columns + host pre-norm.

The reference loss depends only on sims[k, n] where tok[n] == ta[k] and
n != anchor_k.  Sorting positions by token id makes each anchor's
candidate set a contiguous run; the union of runs over the <=256 anchor
tokens covers ~1.3k of the 65k positions ("live" columns).  Each core
gets a contiguous slice of the live-column list (padded to LIVE) plus
the <=128 anchors whose runs intersect its slice, and computes the
dense masked cosine-sim reduction over its slice on device:

  - x   [128, 4, LIVE] f16: normalized live columns (d-block x {r,i})
  - a   [128, 4, 128]  f16: normalized anchors (scaled by 1/|a|)
  - eqm [128, LIVE]    f16 in {0,-3}: 0 = valid other, -3 = invalid /
                        self / dead slot or pad column
  - PE: psum = 4 accumulating f16 matmuls (contraction 256 x {r,i})
  - ACT: c16 = f16 copy of psum (sims)
  - DVE: pacc = c16 + eqm (column-max partial), nacc = c16 - eqm
         (column-min partial); running max/min across chunks
  - out [128, 2, 512] f16 partials -> host

Host combines per-anchor max/min across cores/columns (runs split
across cores are handled by the max/min union) and applies the loss
formula with exact integer index math.  If an input ever needs more
than LIVE live columns per core, a wider program from the size ladder
is compiled (up to the fully dense 8192).
"""

from contextlib import ExitStack

import numpy as np

import concourse.bacc as bacc
import concourse.tile as tile
from concourse import mybir
from concourse.bass_utils import run_bass_kernel_spmd

# ---- problem constants (hardcoded per harness contract) ----
B, S, D = 16, 4096, 256
N = B * S
VOCAB = 16000
KMAX = 256  # MAX_ANCHORS
EPS = 1e-8
TEMPERATURE = 0.1
MARGIN = 1.0
SEPARATION_WEIGHT = 1.0
NCORES = 8

CHUNK = 256
LIVE = 256  # padded live columns per core (harness input needs ~170)
KC = 64  # anchor slots per core (harness input needs <=35)
MASK = 3.0  # additive mask magnitude

F32 = mybir.dt.float32
F16 = mybir.dt.float16

_PROGRAM_CACHE = {}


def build_program(w):
    """Masked-sim partial-reduction program over a width-w column slice."""
    nch = (w + CHUNK - 1) // CHUNK
    assert w % CHUNK == 0
    nc = bacc.Bacc("TRN2", target_bir_lowering=False, debug=False, num_devices=NCORES)
    x_d = nc.dram_tensor("x", [nch, 128, 4, CHUNK], F16, kind="ExternalInput")
    eqm_d = nc.dram_tensor("eqm", [nch, KC, CHUNK], F16, kind="ExternalInput")
    a_d = nc.dram_tensor("a", [128, 4, KC], F16, kind="ExternalInput")
    out_d = nc.dram_tensor("out", [KC, 2, CHUNK], F16, kind="ExternalOutput")

    with tile.TileContext(nc) as tc, ExitStack() as ctx:
        singles = ctx.enter_context(tc.tile_pool(name="singles", bufs=1))
        xpool = ctx.enter_context(tc.tile_pool(name="xpool", bufs=min(nch, 6)))
        eqpool = ctx.enter_context(tc.tile_pool(name="eqpool", bufs=min(nch, 6)))
        scrpool = ctx.enter_context(tc.tile_pool(name="scrpool", bufs=4))
        pspool = ctx.enter_context(tc.tile_pool(name="pspool", bufs=4, space="PSUM"))

        # input DMAs on separate trigger engines so transfers overlap;
        # x (largest, first consumer) goes on sync, which triggers earliest
        acc = singles.tile([KC, 2, CHUNK], F16)
        xts, eqts = [], []
        for ch in range(nch):
            xt = xpool.tile([128, 4, CHUNK], F16, tag="x", name=f"x{ch}")
            nc.sync.dma_start(out=xt, in_=x_d[ch])
            eqt = eqpool.tile([KC, CHUNK], F16, tag="eq", name=f"eq{ch}")
            nc.gpsimd.dma_start(out=eqt, in_=eqm_d[ch])
            xts.append(xt)
            eqts.append(eqt)
        a_sb = singles.tile([128, 4, KC], F16)
        nc.scalar.dma_start(out=a_sb, in_=a_d[:, :, :])

        for ch in range(nch):
            xt, eqt = xts[ch], eqts[ch]
            pst = pspool.tile([KC, CHUNK], F32, tag="ps")
            for b in range(4):
                nc.tensor.matmul(
                    pst,
                    a_sb[:, b, :],
                    xt[:, b, :],
                    start=(b == 0),
                    stop=(b == 3),
                )
            if ch == 0:
                nc.vector.tensor_tensor(
                    out=acc[:, 0, :], in0=pst, in1=eqt, op=mybir.AluOpType.add
                )
                if nch == 1:
                    # overlap the pos-partial writeback with the neg chain
                    nc.sync.dma_start(out=out_d[:, 0, :], in_=acc[:, 0, :])
                nc.vector.tensor_tensor(
                    out=acc[:, 1, :], in0=pst, in1=eqt, op=mybir.AluOpType.subtract
                )
                if nch == 1:
                    nc.scalar.dma_start(out=out_d[:, 1, :], in_=acc[:, 1, :])
            else:
                scr = scrpool.tile([KC, CHUNK], F16, tag="scr")
                nc.vector.tensor_tensor(
                    out=scr, in0=pst, in1=eqt, op=mybir.AluOpType.add
                )
                nc.vector.tensor_tensor(
                    out=acc[:, 0, :], in0=acc[:, 0, :], in1=scr,
                    op=mybir.AluOpType.max,
                )
                scr2 = scrpool.tile([KC, CHUNK], F16, tag="scr2")
                nc.vector.tensor_tensor(
                    out=scr2, in0=pst, in1=eqt, op=mybir.AluOpType.subtract
                )
                nc.vector.tensor_tensor(
                    out=acc[:, 1, :], in0=acc[:, 1, :], in1=scr2,
                    op=mybir.AluOpType.min,
                )
        if nch > 1:
            nc.sync.dma_start(out=out_d[:, :, :], in_=acc)

    nc.compile()
    return nc


def host_prep(real_embeds, imag_embeds, token_ids):
    """Live-column selection, normalization, per-core input build."""
    Rf = np.asarray(real_embeds, dtype=np.float32).reshape(N, D)
    If = np.asarray(imag_embeds, dtype=np.float32).reshape(N, D)
    tok = np.asarray(token_ids).reshape(N).astype(np.int64, copy=False)

    counts = np.bincount(tok, minlength=VOCAB)
    repeated = counts[tok] >= 2
    order = np.argsort(~repeated, kind="stable")
    anchors = order[:KMAX]
    anchor_ok = repeated[anchors]
    ta = tok[anchors]
    num_others = counts[ta] - 1
    pair_ok = anchor_ok & (num_others >= 2)

    perm = np.argsort(tok, kind="stable")
    tok_s = tok[perm]

    # live columns: sorted positions whose token is an anchor token
    live_mask = np.zeros(VOCAB, dtype=bool)
    live_mask[ta] = True
    live_idx = np.nonzero(live_mask[tok_s])[0]  # indices into sorted order
    n_live = len(live_idx)
    per_core = -(-n_live // NCORES)  # ceil split
    w = LIVE
    while per_core > w:
        w *= 2
    assert w <= N // NCORES

    live_cols = perm[live_idx]  # original position of each live column
    live_tok = tok_s[live_idx]

    # normalize only what the device needs: live columns + anchors
    need = np.concatenate([live_cols, anchors])
    inv_need = 1.0 / np.sqrt(
        np.einsum("nd,nd->n", Rf[need], Rf[need])
        + np.einsum("nd,nd->n", If[need], If[need])
        + EPS
    )
    inv_live = inv_need[:n_live].astype(np.float32)
    inv_anc = inv_need[n_live:].astype(np.float32)

    nch = w // CHUNK
    in_maps = []
    slot_maps = []
    for c in range(NCORES):
        lo = min(c * per_core, n_live)
        hi = min(lo + per_core, n_live)
        cols = live_cols[lo:hi]
        ctok = live_tok[lo:hi]
        m = len(cols)

        xw = np.zeros((128, 4, w), dtype=np.float16)
        if m:
            Rn = (Rf[cols] * inv_live[lo:hi][:, None]).astype(np.float16)
            In = (If[cols] * inv_live[lo:hi][:, None]).astype(np.float16)
            xw[:, 0, :m] = Rn.T[:128]
            xw[:, 1, :m] = Rn.T[128:]
            xw[:, 2, :m] = In.T[:128]
            xw[:, 3, :m] = In.T[128:]
        x = np.ascontiguousarray(
            xw.reshape(128, 4, nch, CHUNK).transpose(2, 0, 1, 3)
        )

        # anchors owned: any live column here carries their token
        own = np.nonzero(np.isin(ta, ctok))[0] if m else np.array([], dtype=int)
        assert len(own) <= KC, f"core {c}: {len(own)} anchors > {KC} slots"
        slot_maps.append(own)

        a = np.zeros((128, 4, KC), dtype=np.float16)
        if len(own):
            Ra = Rf[anchors[own]] * inv_anc[own][:, None]
            Ia = If[anchors[own]] * inv_anc[own][:, None]
            a[:, 0, : len(own)] = Ra.T[:128].astype(np.float16)
            a[:, 1, : len(own)] = Ra.T[128:].astype(np.float16)
            a[:, 2, : len(own)] = Ia.T[:128].astype(np.float16)
            a[:, 3, : len(own)] = Ia.T[128:].astype(np.float16)

        eqm = np.full((KC, w), -MASK, dtype=np.float16)
        if len(own):
            valid = ta[own][:, None] == ctok[None, :]
            # self-exclusion: an anchor's own position is not an "other"
            selfcol = anchors[own][:, None] == cols[None, :]
            eqm[: len(own), :m] = np.where(valid & ~selfcol, 0.0, -MASK)
        eqm = np.ascontiguousarray(eqm.reshape(KC, nch, CHUNK).transpose(1, 0, 2))

        in_maps.append({"x": x, "eqm": eqm, "a": a})

    meta = {"pair_ok": pair_ok, "slot_maps": slot_maps, "w": w}
    return in_maps, meta


def combine(results, meta):
    pos = np.full(KMAX, -np.inf, dtype=np.float64)
    neg = np.full(KMAX, np.inf, dtype=np.float64)
    for c, res in enumerate(results):
        o = np.asarray(res["out"], dtype=np.float64)  # [128, 2, CHUNK]
        own = meta["slot_maps"][c]
        if len(own) == 0:
            continue
        p = o[: len(own), 0, :].max(axis=1)
        q = o[: len(own), 1, :].min(axis=1)
        np.maximum.at(pos, own, p)
        np.minimum.at(neg, own, q)

    pair_ok = meta["pair_ok"]
    num_pairs = int(pair_ok.sum())
    if num_pairs == 0:
        return np.float32(0.0)
    lp = pos / TEMPERATURE
    ln = neg / TEMPERATURE
    m = np.maximum(lp, ln)
    lse = m + np.log(np.exp(lp - m) + np.exp(ln - m))
    ce = lse - lp
    sep = np.maximum(neg + MARGIN, 0.0)
    per_anchor = ce + SEPARATION_WEIGHT * sep
    total = float(np.sum(per_anchor[pair_ok]))
    return np.float32(total / num_pairs)


def kernel_with_results(real_embeds, imag_embeds, token_ids, trace=False):
    in_maps, meta = host_prep(real_embeds, imag_embeds, token_ids)
    w = meta["w"]
    if w not in _PROGRAM_CACHE:
        _PROGRAM_CACHE[w] = build_program(w)
    nc = _PROGRAM_CACHE[w]
    br = run_bass_kernel_spmd(nc, in_maps, core_ids=list(range(NCORES)), trace=trace)
    loss = combine(br.results, meta)
    return loss, br


def kernel(real_embeds, imag_embeds, token_ids):
    loss, _ = kernel_with_results(real_embeds, imag_embeds, token_ids)
    return loss


# revision 18
# speedup vs baseline: 1.0080x; 1.0073x over previous
"""Trainium2 Bass kernel for nn_ContrastivePhaseObjective.

Strategy: token-locality sharding over LIVE columns + host pre-norm.

The reference loss depends only on sims[k, n] where tok[n] == ta[k] and
n != anchor_k.  Sorting positions by token id makes each anchor's
candidate set a contiguous run; the union of runs over the <=256 anchor
tokens covers ~1.3k of the 65k positions ("live" columns).  Each core
gets a contiguous slice of the live-column list (padded to LIVE) plus
the <=128 anchors whose runs intersect its slice, and computes the
dense masked cosine-sim reduction over its slice on device:

  - x   [128, 4, LIVE] f16: normalized live columns (d-block x {r,i})
  - a   [128, 4, 128]  f16: normalized anchors (scaled by 1/|a|)
  - eqm [128, LIVE]    f16 in {0,-3}: 0 = valid other, -3 = invalid /
                        self / dead slot or pad column
  - PE: psum = 4 accumulating f16 matmuls (contraction 256 x {r,i})
  - ACT: c16 = f16 copy of psum (sims)
  - DVE: pacc = c16 + eqm (column-max partial), nacc = c16 - eqm
         (column-min partial); running max/min across chunks
  - out [128, 2, 512] f16 partials -> host

Host combines per-anchor max/min across cores/columns (runs split
across cores are handled by the max/min union) and applies the loss
formula with exact integer index math.  If an input ever needs more
than LIVE live columns per core, a wider program from the size ladder
is compiled (up to the fully dense 8192).
"""

from contextlib import ExitStack

import numpy as np

import concourse.bacc as bacc
import concourse.tile as tile
from concourse import mybir
from concourse.bass_utils import run_bass_kernel_spmd

# ---- problem constants (hardcoded per harness contract) ----
B, S, D = 16, 4096, 256
N = B * S
VOCAB = 16000
KMAX = 256  # MAX_ANCHORS
EPS = 1e-8
TEMPERATURE = 0.1
MARGIN = 1.0
SEPARATION_WEIGHT = 1.0
NCORES = 8

CHUNK = 256
LIVE = 256  # padded live columns per core (harness input needs ~170)
KC = 64  # anchor slots per core (harness input needs <=35)
MASK = 3.0  # additive mask magnitude

F32 = mybir.dt.float32
F16 = mybir.dt.float16

_PROGRAM_CACHE = {}


def build_program(w):
    """Masked-sim partial-reduction program over a width-w column slice."""
    nch = (w + CHUNK - 1) // CHUNK
    assert w % CHUNK == 0
    nc = bacc.Bacc("TRN2", target_bir_lowering=False, debug=False, num_devices=NCORES)
    xr_d = nc.dram_tensor("xr", [nch, 128, 2, CHUNK], F16, kind="ExternalInput")
    xi_d = nc.dram_tensor("xi", [nch, 128, 2, CHUNK], F16, kind="ExternalInput")
    # anchors + all eqm chunks in one blob (eqm on partitions 0..KC-1)
    aeq_d = nc.dram_tensor(
        "aeq", [128, 4 * KC + nch * CHUNK], F16, kind="ExternalInput"
    )
    out_d = nc.dram_tensor("out", [KC, 2, CHUNK], F16, kind="ExternalOutput")

    with tile.TileContext(nc) as tc, ExitStack() as ctx:
        singles = ctx.enter_context(tc.tile_pool(name="singles", bufs=1))
        xpool = ctx.enter_context(tc.tile_pool(name="xpool", bufs=min(2 * nch, 8)))
        scrpool = ctx.enter_context(tc.tile_pool(name="scrpool", bufs=4))
        pspool = ctx.enter_context(tc.tile_pool(name="pspool", bufs=4, space="PSUM"))

        # three parallel input DMA paths: x halves on sync/scalar (earliest
        # triggers), anchors+masks on gpsimd
        acc = singles.tile([KC, 2, CHUNK], F16)
        xrts, xits = [], []
        for ch in range(nch):
            xrt = xpool.tile([128, 2, CHUNK], F16, tag="xr", name=f"xr{ch}")
            nc.sync.dma_start(out=xrt, in_=xr_d[ch])
            xit = xpool.tile([128, 2, CHUNK], F16, tag="xi", name=f"xi{ch}")
            nc.scalar.dma_start(out=xit, in_=xi_d[ch])
            xrts.append(xrt)
            xits.append(xit)
        aeqt = singles.tile([128, 4 * KC + nch * CHUNK], F16)
        nc.gpsimd.dma_start(out=aeqt, in_=aeq_d[:, :])

        for ch in range(nch):
            eqt = aeqt[0:KC, 4 * KC + ch * CHUNK : 4 * KC + (ch + 1) * CHUNK]
            pst = pspool.tile([KC, CHUNK], F32, tag="ps")
            for b in range(4):
                xt = xrts[ch] if b < 2 else xits[ch]
                nc.tensor.matmul(
                    pst,
                    aeqt[:, b * KC : (b + 1) * KC],
                    xt[:, b % 2, :],
                    start=(b == 0),
                    stop=(b == 3),
                )
            if ch == 0:
                nc.vector.tensor_tensor(
                    out=acc[:, 0, :], in0=pst, in1=eqt, op=mybir.AluOpType.add
                )
                if nch == 1:
                    # overlap the pos-partial writeback with the neg chain
                    nc.sync.dma_start(out=out_d[:, 0, :], in_=acc[:, 0, :])
                nc.vector.tensor_tensor(
                    out=acc[:, 1, :], in0=pst, in1=eqt, op=mybir.AluOpType.subtract
                )
                if nch == 1:
                    nc.scalar.dma_start(out=out_d[:, 1, :], in_=acc[:, 1, :])
            else:
                scr = scrpool.tile([KC, CHUNK], F16, tag="scr")
                nc.vector.tensor_tensor(
                    out=scr, in0=pst, in1=eqt, op=mybir.AluOpType.add
                )
                nc.vector.tensor_tensor(
                    out=acc[:, 0, :], in0=acc[:, 0, :], in1=scr,
                    op=mybir.AluOpType.max,
                )
                scr2 = scrpool.tile([KC, CHUNK], F16, tag="scr2")
                nc.vector.tensor_tensor(
                    out=scr2, in0=pst, in1=eqt, op=mybir.AluOpType.subtract
                )
                nc.vector.tensor_tensor(
                    out=acc[:, 1, :], in0=acc[:, 1, :], in1=scr2,
                    op=mybir.AluOpType.min,
                )
        if nch > 1:
            nc.sync.dma_start(out=out_d[:, :, :], in_=acc)

    nc.compile()
    return nc


def host_prep(real_embeds, imag_embeds, token_ids):
    """Live-column selection, normalization, per-core input build."""
    Rf = np.asarray(real_embeds, dtype=np.float32).reshape(N, D)
    If = np.asarray(imag_embeds, dtype=np.float32).reshape(N, D)
    tok = np.asarray(token_ids).reshape(N).astype(np.int64, copy=False)

    counts = np.bincount(tok, minlength=VOCAB)
    repeated = counts[tok] >= 2
    order = np.argsort(~repeated, kind="stable")
    anchors = order[:KMAX]
    anchor_ok = repeated[anchors]
    ta = tok[anchors]
    num_others = counts[ta] - 1
    pair_ok = anchor_ok & (num_others >= 2)

    perm = np.argsort(tok, kind="stable")
    tok_s = tok[perm]

    # live columns: sorted positions whose token is an anchor token
    live_mask = np.zeros(VOCAB, dtype=bool)
    live_mask[ta] = True
    live_idx = np.nonzero(live_mask[tok_s])[0]  # indices into sorted order
    n_live = len(live_idx)
    per_core = -(-n_live // NCORES)  # ceil split
    w = LIVE
    while per_core > w:
        w *= 2
    assert w <= N // NCORES

    live_cols = perm[live_idx]  # original position of each live column
    live_tok = tok_s[live_idx]

    # normalize only what the device needs: live columns + anchors
    need = np.concatenate([live_cols, anchors])
    inv_need = 1.0 / np.sqrt(
        np.einsum("nd,nd->n", Rf[need], Rf[need])
        + np.einsum("nd,nd->n", If[need], If[need])
        + EPS
    )
    inv_live = inv_need[:n_live].astype(np.float32)
    inv_anc = inv_need[n_live:].astype(np.float32)

    nch = w // CHUNK
    in_maps = []
    slot_maps = []
    for c in range(NCORES):
        lo = min(c * per_core, n_live)
        hi = min(lo + per_core, n_live)
        cols = live_cols[lo:hi]
        ctok = live_tok[lo:hi]
        m = len(cols)

        xrw = np.zeros((128, 2, w), dtype=np.float16)
        xiw = np.zeros((128, 2, w), dtype=np.float16)
        if m:
            Rn = (Rf[cols] * inv_live[lo:hi][:, None]).astype(np.float16)
            In = (If[cols] * inv_live[lo:hi][:, None]).astype(np.float16)
            xrw[:, 0, :m] = Rn.T[:128]
            xrw[:, 1, :m] = Rn.T[128:]
            xiw[:, 0, :m] = In.T[:128]
            xiw[:, 1, :m] = In.T[128:]
        xr = np.ascontiguousarray(
            xrw.reshape(128, 2, nch, CHUNK).transpose(2, 0, 1, 3)
        )
        xi = np.ascontiguousarray(
            xiw.reshape(128, 2, nch, CHUNK).transpose(2, 0, 1, 3)
        )

        # anchors owned: any live column here carries their token
        own = np.nonzero(np.isin(ta, ctok))[0] if m else np.array([], dtype=int)
        assert len(own) <= KC, f"core {c}: {len(own)} anchors > {KC} slots"
        slot_maps.append(own)

        aeq = np.zeros((128, 4 * KC + w), dtype=np.float16)
        if len(own):
            Ra = Rf[anchors[own]] * inv_anc[own][:, None]
            Ia = If[anchors[own]] * inv_anc[own][:, None]
            aeq[:, 0 * KC : 0 * KC + len(own)] = Ra.T[:128].astype(np.float16)
            aeq[:, 1 * KC : 1 * KC + len(own)] = Ra.T[128:].astype(np.float16)
            aeq[:, 2 * KC : 2 * KC + len(own)] = Ia.T[:128].astype(np.float16)
            aeq[:, 3 * KC : 3 * KC + len(own)] = Ia.T[128:].astype(np.float16)

        eqm = np.full((KC, w), -MASK, dtype=np.float16)
        if len(own):
            valid = ta[own][:, None] == ctok[None, :]
            # self-exclusion: an anchor's own position is not an "other"
            selfcol = anchors[own][:, None] == cols[None, :]
            eqm[: len(own), :m] = np.where(valid & ~selfcol, 0.0, -MASK)
        aeq[:KC, 4 * KC :] = eqm

        in_maps.append({"xr": xr, "xi": xi, "aeq": aeq})

    meta = {"pair_ok": pair_ok, "slot_maps": slot_maps, "w": w}
    return in_maps, meta


def combine(results, meta):
    pos = np.full(KMAX, -np.inf, dtype=np.float64)
    neg = np.full(KMAX, np.inf, dtype=np.float64)
    for c, res in enumerate(results):
        o = np.asarray(res["out"], dtype=np.float64)  # [128, 2, CHUNK]
        own = meta["slot_maps"][c]
        if len(own) == 0:
            continue
        p = o[: len(own), 0, :].max(axis=1)
        q = o[: len(own), 1, :].min(axis=1)
        np.maximum.at(pos, own, p)
        np.minimum.at(neg, own, q)

    pair_ok = meta["pair_ok"]
    num_pairs = int(pair_ok.sum())
    if num_pairs == 0:
        return np.float32(0.0)
    lp = pos / TEMPERATURE
    ln = neg / TEMPERATURE
    m = np.maximum(lp, ln)
    lse = m + np.log(np.exp(lp - m) + np.exp(ln - m))
    ce = lse - lp
    sep = np.maximum(neg + MARGIN, 0.0)
    per_anchor = ce + SEPARATION_WEIGHT * sep
    total = float(np.sum(per_anchor[pair_ok]))
    return np.float32(total / num_pairs)


def kernel_with_results(real_embeds, imag_embeds, token_ids, trace=False):
    in_maps, meta = host_prep(real_embeds, imag_embeds, token_ids)
    w = meta["w"]
    if w not in _PROGRAM_CACHE:
        _PROGRAM_CACHE[w] = build_program(w)
    nc = _PROGRAM_CACHE[w]
    br = run_bass_kernel_spmd(nc, in_maps, core_ids=list(range(NCORES)), trace=trace)
    loss = combine(br.results, meta)
    return loss, br


def kernel(real_embeds, imag_embeds, token_ids):
    loss, _ = kernel_with_results(real_embeds, imag_embeds, token_ids)
    return loss
